# revision 9
# baseline (speedup 1.0000x reference)
# Trainium2 Bass kernel for masked causal attention
#   B=2, H=16, S=2048, D=64, bool attn_mask [B, S, S] + causal, softmax, @V.
#
# Sharding: 8 cores x 4 heads (cores 0-3 -> batch 0, cores 4-7 -> batch 1).
# Each core computes its 4 heads fully on-device; the per-batch mask is
# resident in SBUF and shared by the core's 4 heads.
#
# Per (head, k-tile kt of 128 keys):
#   S^T[k, q] = sum_d K[k,d] Q[q,d]     (PE: lhsT=K^T tile, rhs=Q^T, fp16)
#   p[k, q]   = exp(S^T/8) * mask^T     (ACT exp -> DVE mask-mult for most
#                                        tiles; every SCHRAUD'th tile runs a
#                                        Schraudolph exp2 bit-trick fully on
#                                        DVE: i16 = int16(s*C+B); p =
#                                        bitcast_fp16(i16) * mask -- offloads
#                                        the ACT engine, the pointwise
#                                        bottleneck)
#   outT[m,q] += sum_k vp[k,m] p[k,q]   (PE: lhsT=[V | ones] -> row 64 = denom)
# The normalization (num/den) happens ON HOST: the kernel ships the
# unnormalized [65, S] accumulator as fp16, killing the on-chip
# reciprocal/broadcast/divide tail entirely.
# Causal structure is exploited exactly: k-tile kt only computes q >= 128*kt.

import os
import numpy as np

B, H, S, D = 2, 16, 2048, 64
NCORES = 8
HPC = 4          # heads per core
P = 128
NKT = S // P     # 16 k-tiles
CHUNK = 1024     # q-chunk size for the S^T psum tile (2 PSUM banks)
# every SCHRAUD-th softmax tile (by per-head chunk index) takes the DVE
# Schraudolph path instead of ACT exp. 0 disables.
SCHRAUD = int(os.environ.get("ATTN_SCHRAUD", "5"))
PV_DELAY = int(os.environ.get("ATTN_PV_DELAY", "6"))

# Schraudolph constants for p = exp(s * 0.125) via fp16 bit pattern:
#   i16 = int16(s * C_S + B_S);  p = bitcast_fp16(i16)
# C = 0.125 * 2^10 / ln2, B = 15*2^10 - c with c=44 centering the relative
# error of the linear-mantissa approximation (max ~3.1%, rms ~2.1%).
C_SCH = float(0.125 * 1024.0 / np.log(2.0))
B_SCH = float(15.0 * 1024.0 - 44.0)

_cache = {}


def build_nc():
    import concourse.bacc as bacc
    import concourse.mybir as mybir
    import concourse.tile as tile
    from contextlib import ExitStack

    fp16 = mybir.dt.float16
    f32 = mybir.dt.float32
    i16 = mybir.dt.int16
    Exp = mybir.ActivationFunctionType.Exp
    Copy = mybir.ActivationFunctionType.Copy
    Mult = mybir.AluOpType.mult
    Add = mybir.AluOpType.add

    nc = bacc.Bacc("TRN2", target_bir_lowering=False, debug=False,
                   num_devices=NCORES)

    # Host-prepared, per-core inputs.
    qt_d = nc.dram_tensor("qt", [HPC, 64, S], fp16, kind="ExternalInput")
    kt_d = nc.dram_tensor("kt", [HPC, 64, S], fp16, kind="ExternalInput")
    vp_d = nc.dram_tensor("vp", [HPC, P, NKT, D + 1], fp16, kind="ExternalInput")
    mk_d = nc.dram_tensor("maskt", [P, NKT, S], fp16, kind="ExternalInput")
    # unnormalized output: rows 0..63 = numerator^T, row 64 = denominator
    out_d = nc.dram_tensor("outt", [HPC, D + 1, S], fp16, kind="ExternalOutput")

    with tile.TileContext(nc) as tc, ExitStack() as ctx:
        mask_pool = ctx.enter_context(tc.tile_pool(name="mask", bufs=1))
        qk_pool = ctx.enter_context(tc.tile_pool(name="qk", bufs=2))
        vp_pool = ctx.enter_context(tc.tile_pool(name="vpool", bufs=2))
        p_pool = ctx.enter_context(tc.tile_pool(name="p", bufs=8))
        s_pool = ctx.enter_context(tc.tile_pool(name="sch", bufs=4))
        o_pool = ctx.enter_context(tc.tile_pool(name="osb", bufs=4))
        st_psum = ctx.enter_context(tc.tile_pool(name="st", bufs=2, space="PSUM"))
        o_psum = ctx.enter_context(tc.tile_pool(name="outp", bufs=1, space="PSUM"))

        def load_head(h):
            qt = qk_pool.tile([64, S], fp16, tag="qt")
            kt = qk_pool.tile([64, S], fp16, tag="kt")
            vp = vp_pool.tile([P, NKT, D + 1], fp16, tag="vp")
            nc.sync.dma_start(qt[:], qt_d[h])
            nc.sync.dma_start(kt[:], kt_d[h])
            nc.sync.dma_start(vp[:], vp_d[h])
            return qt, kt, vp

        # Start-latency-ordered first loads: the j=0 QK chunk unblocks on
        # ~80KB (kt k-tile 0 + first 512 q's), then mask plane 0 for the
        # first softmax, then the rest of head 0; the remaining mask planes
        # stream in behind, one per k-tile.  Causal trim: k-tile g's mask is
        # only ever read for q >= 128g, so skip the lower-triangle bytes
        # (-47% mask traffic).
        qt0 = qk_pool.tile([64, S], fp16, tag="qt")
        kt0 = qk_pool.tile([64, S], fp16, tag="kt")
        vp0 = vp_pool.tile([P, NKT, D + 1], fp16, tag="vp")
        mask_sb = mask_pool.tile([P, NKT, S], fp16, tag="mask")
        nc.sync.dma_start(kt0[:, 0:P], kt_d[0, :, 0:P])
        nc.sync.dma_start(qt0[:, 0:512], qt_d[0, :, 0:512])
        nc.sync.dma_start(mask_sb[:, 0:1, :], mk_d[:, 0:1, :])
        nc.sync.dma_start(qt0[:, 512:CHUNK], qt_d[0, :, 512:CHUNK])
        nc.sync.dma_start(kt0[:, P:], kt_d[0, :, P:])
        nc.sync.dma_start(qt0[:, CHUNK:], qt_d[0, :, CHUNK:])
        nc.sync.dma_start(vp0[:], vp_d[0])
        head_tiles = {0: (qt0, kt0, vp0)}
        for g in range(1, NKT):
            c0 = g * P
            nc.sync.dma_start(mask_sb[:, g:g + 1, c0:], mk_d[:, g:g + 1, c0:])

        for h in range(HPC):
            qt, kt, vp = head_tiles.pop(h, None) or load_head(h)
            outp = o_psum.tile([D + 1, S], f32, tag="outp")
            pending_pv = []

            def emit_evac(b):
                # bank b of outp ([65, 512] f32) is fully accumulated ->
                # convert to fp16 in SBUF and ship; host divides num/den.
                s0, s1 = 512 * b, 512 * (b + 1)
                osb = o_pool.tile([D + 1, 512], fp16, tag="osb")
                if b % 2 == 0:
                    # Copy lives in the same ACT table as Exp: no table reload
                    nc.scalar.activation(osb[:], outp[:, s0:s1], Copy)
                else:
                    nc.vector.tensor_copy(osb[:], outp[:, s0:s1])
                nc.sync.dma_start(out_d[h, :, s0:s1], osb[:])

            def emit_pv(j, c, e, p):
                for b in range(c // 512, (e + 511) // 512):
                    g0, g1 = max(c, 512 * b), min(e, 512 * (b + 1))
                    nc.tensor.matmul(outp[:, g0:g1], lhsT=vp[:, j, :],
                                     rhs=p[:, g0 - c:g1 - c],
                                     start=(j == 0),
                                     stop=(j == min(4 * b + 3, NKT - 1)))
                # bank b=(j-3)//4 is fully accumulated once k-tile j=4b+3's
                # last chunk (e == S) has been emitted
                if e == S and j % 4 == 3:
                    emit_evac((j - 3) // 4)

            def chunks(j):
                out, c = [], j * P
                while c < S:
                    e = min(S, (c // CHUNK + 1) * CHUNK)
                    out.append((c, e))
                    c = e
                return out

            t_idx = 0

            def emit_softmax(j, c, e, stt):
                nonlocal t_idx
                w = e - c
                p = p_pool.tile([P, CHUNK], fp16, tag="p")
                if SCHRAUD and t_idx % SCHRAUD == SCHRAUD // 2:
                    # full-DVE path: exp2 bit trick + fused mask multiply
                    isch = s_pool.tile([P, CHUNK], i16, tag="isch")
                    nc.vector.tensor_scalar(isch[:, :w], stt[:, :w],
                                            C_SCH, B_SCH, Mult, Add)
                    nc.vector.tensor_tensor(p[:, :w],
                                            isch[:, :w].bitcast(fp16),
                                            mask_sb[:, j, c:c + w], Mult)
                else:
                    nc.scalar.activation(p[:, :w], stt[:, :w], Exp, scale=0.125)
                    nc.vector.tensor_mul(p[:, :w], p[:, :w],
                                         mask_sb[:, j, c:c + w])
                t_idx += 1
                pending_pv.append((j, c, e, p))
                # drain PVs two at a time: each QK<->PV switch costs ~140ns
                # of PE row-config drain, so batching same-kind matmuls
                # halves the number of switches
                if len(pending_pv) >= PV_DELAY + 2:
                    emit_pv(*pending_pv.pop(0))
                    emit_pv(*pending_pv.pop(0))

            for j in range(NKT):
                # prefetch the next head's inputs mid-head so their DMAs
                # finish well before the head boundary (the tail of this
                # head still has ~8 k-tiles of PE work to hide them behind)
                if j == 8 and h + 1 < HPC:
                    head_tiles[h + 1] = load_head(h + 1)
                lhs = kt[:, j * P:(j + 1) * P]
                for c, e in chunks(j):
                    stt = st_psum.tile([P, CHUNK], f32, tag="st")
                    for lo in range(0, e - c, 512):
                        wl = min(512, e - c - lo)
                        nc.tensor.matmul(stt[:, lo:lo + wl], lhsT=lhs,
                                         rhs=qt[:, c + lo:c + lo + wl],
                                         start=True, stop=True)
                    emit_softmax(j, c, e, stt)

            while pending_pv:
                emit_pv(*pending_pv.pop(0))

    nc.compile()
    return nc


def prep_inputs(query, key, value, attn_mask):
    """Host-side layout prep (transposes/retiling/casts only) -> 8 in_maps."""
    query = np.asarray(query, dtype=np.float32)
    key = np.asarray(key, dtype=np.float32)
    value = np.asarray(value, dtype=np.float32)
    attn_mask = np.asarray(attn_mask).astype(bool)

    qT = np.ascontiguousarray(query.transpose(0, 1, 3, 2)).astype(np.float16)
    kT = np.ascontiguousarray(key.transpose(0, 1, 3, 2)).astype(np.float16)

    vp = np.concatenate(
        [value, np.ones((B, H, S, 1), np.float32)], axis=3).astype(np.float16)
    # [B, H, S, 65] -> [B, H, 128, NKT, 65] (partition-contiguous tiles)
    vp = np.ascontiguousarray(
        vp.reshape(B, H, NKT, P, D + 1).transpose(0, 1, 3, 2, 4))

    tril = np.tril(np.ones((S, S), dtype=bool))
    in_maps = []
    for b in range(B):
        m = (attn_mask[b] & tril)          # [q, k]
        mT = m.T.astype(np.float16)        # [k, q]
        maskt = np.ascontiguousarray(
            mT.reshape(NKT, P, S).transpose(1, 0, 2))  # [128, NKT, S]
        for cl in range(NCORES // B):
            h0 = cl * HPC
            in_maps.append({
                "qt": np.ascontiguousarray(qT[b, h0:h0 + HPC]),
                "kt": np.ascontiguousarray(kT[b, h0:h0 + HPC]),
                "vp": np.ascontiguousarray(vp[b, h0:h0 + HPC]),
                "maskt": maskt,
            })
    return in_maps


def run(query, key, value, attn_mask, trace=False, trace_cores=None):
    from concourse import bass_utils

    if "nc" not in _cache:
        _cache["nc"] = build_nc()
    nc = _cache["nc"]

    in_maps = prep_inputs(query, key, value, attn_mask)
    res = bass_utils.run_bass_kernel_spmd(
        nc, in_maps, core_ids=list(range(NCORES)),
        trace=trace, trace_cores=trace_cores)

    out = np.empty((B, H, S, D), np.float32)
    for c in range(NCORES):
        b = c // (NCORES // B)
        h0 = (c % (NCORES // B)) * HPC
        outt = res.results[c]["outt"].astype(np.float32)   # [HPC, 65, S]
        num = outt[:, 0:D, :]                              # [HPC, 64, S]
        den = outt[:, D:D + 1, :]                          # [HPC, 1, S]
        out[b, h0:h0 + HPC] = (num / den).transpose(0, 2, 1)
    return out, res


def kernel(query, key, value, attn_mask):
    out, _ = run(query, key, value, attn_mask)
    return out


# revision 13
# speedup vs baseline: 1.4606x; 1.4606x over previous
# Trainium2 Bass kernel for masked causal attention
#   B=2, H=16, S=2048, D=64, bool attn_mask [B, S, S] + causal, softmax, @V.
#
# Sharding: 8 cores x 4 heads (cores 0-3 -> batch 0, cores 4-7 -> batch 1).
# Each core computes its 4 heads fully on-device; the per-batch mask is
# resident in SBUF and shared by the core's 4 heads.
#
# Per (head, k-tile kt of 128 keys):
#   S^T[k, q] = sum_d K[k,d] Q[q,d]     (PE: lhsT=K^T tile, rhs=Q^T, fp16)
#   p[k, q]   = exp(S^T/8) * mask^T     (ACT exp -> DVE mask-mult for most
#                                        tiles; every SCHRAUD'th tile runs a
#                                        Schraudolph exp2 bit-trick fully on
#                                        DVE: i16 = int16(s*C+B); p =
#                                        bitcast_fp16(i16) * mask -- offloads
#                                        the ACT engine, the pointwise
#                                        bottleneck)
#   outT[m,q] += sum_k vp[k,m] p[k,q]   (PE: lhsT=[V | ones] -> row 64 = denom)
# The normalization (num/den) happens ON HOST: the kernel ships the
# unnormalized [65, S] accumulator as fp16, killing the on-chip
# reciprocal/broadcast/divide tail entirely.
# Causal structure is exploited exactly: k-tile kt only computes q >= 128*kt.

import os
import numpy as np

B, H, S, D = 2, 16, 2048, 64
NCORES = 8
HPC = 4          # heads per core
P = 128
NKT = S // P     # 16 k-tiles
CHUNK = 1024     # q-chunk size for the S^T psum tile (2 PSUM banks)
# every SCHRAUD-th softmax tile (by per-head chunk index) takes the DVE
# Schraudolph path instead of ACT exp. 0 disables.
SCHRAUD = int(os.environ.get("ATTN_SCHRAUD", "5"))
PV_DELAY = int(os.environ.get("ATTN_PV_DELAY", "6"))

# Schraudolph constants for p = exp(s * 0.125) via fp16 bit pattern:
#   i16 = int16(s * C_S + B_S);  p = bitcast_fp16(i16)
# C = 0.125 * 2^10 / ln2, B = 15*2^10 - c with c=44 centering the relative
# error of the linear-mantissa approximation (max ~3.1%, rms ~2.1%).
C_SCH = float(0.125 * 1024.0 / np.log(2.0))
B_SCH = float(15.0 * 1024.0 - 44.0)

_cache = {}


def build_nc():
    import concourse.bacc as bacc
    import concourse.mybir as mybir
    import concourse.tile as tile
    from contextlib import ExitStack

    fp16 = mybir.dt.float16
    f32 = mybir.dt.float32
    i16 = mybir.dt.int16
    Exp = mybir.ActivationFunctionType.Exp
    Copy = mybir.ActivationFunctionType.Copy
    Mult = mybir.AluOpType.mult
    Add = mybir.AluOpType.add

    nc = bacc.Bacc("TRN2", target_bir_lowering=False, debug=False,
                   num_devices=NCORES)

    # Host-prepared, per-core inputs.  q^T/k^T are zero-padded from 64 to 128
    # partition rows: the QK matmul then runs in the PE's 128-row config --
    # the same config as PV -- so the ~140ns row-config drain on every
    # QK<->PV transition (~20us/core) disappears.  Zero rows add nothing
    # numerically and columns stream at 1 col/cycle regardless of rows.
    qt_d = nc.dram_tensor("qt", [HPC, P, S], fp16, kind="ExternalInput")
    kt_d = nc.dram_tensor("kt", [HPC, P, S], fp16, kind="ExternalInput")
    vp_d = nc.dram_tensor("vp", [HPC, P, NKT, D + 1], fp16, kind="ExternalInput")
    mk_d = nc.dram_tensor("maskt", [P, NKT, S], fp16, kind="ExternalInput")
    # unnormalized output: rows 0..63 = numerator^T, row 64 = denominator
    out_d = nc.dram_tensor("outt", [HPC, D + 1, S], fp16, kind="ExternalOutput")

    with tile.TileContext(nc) as tc, ExitStack() as ctx:
        mask_pool = ctx.enter_context(tc.tile_pool(name="mask", bufs=1))
        qk_pool = ctx.enter_context(tc.tile_pool(name="qk", bufs=2))
        vp_pool = ctx.enter_context(tc.tile_pool(name="vpool", bufs=2))
        p_pool = ctx.enter_context(tc.tile_pool(name="p", bufs=8))
        s_pool = ctx.enter_context(tc.tile_pool(name="sch", bufs=4))
        o_pool = ctx.enter_context(tc.tile_pool(name="osb", bufs=4))
        st_psum = ctx.enter_context(tc.tile_pool(name="st", bufs=2, space="PSUM"))
        o_psum = ctx.enter_context(tc.tile_pool(name="outp", bufs=1, space="PSUM"))

        def load_head(h):
            qt = qk_pool.tile([P, S], fp16, tag="qt")
            kt = qk_pool.tile([P, S], fp16, tag="kt")
            vp = vp_pool.tile([P, NKT, D + 1], fp16, tag="vp")
            nc.sync.dma_start(qt[:], qt_d[h])
            nc.sync.dma_start(kt[:], kt_d[h])
            nc.sync.dma_start(vp[:], vp_d[h])
            return qt, kt, vp

        # Start-latency-ordered first loads: the j=0 QK chunk unblocks on
        # ~80KB (kt k-tile 0 + first 512 q's), then mask plane 0 for the
        # first softmax, then the rest of head 0; the remaining mask planes
        # stream in behind, one per k-tile.  Causal trim: k-tile g's mask is
        # only ever read for q >= 128g, so skip the lower-triangle bytes
        # (-47% mask traffic).
        qt0 = qk_pool.tile([P, S], fp16, tag="qt")
        kt0 = qk_pool.tile([P, S], fp16, tag="kt")
        vp0 = vp_pool.tile([P, NKT, D + 1], fp16, tag="vp")
        mask_sb = mask_pool.tile([P, NKT, S], fp16, tag="mask")
        nc.sync.dma_start(kt0[:, 0:P], kt_d[0, :, 0:P])
        nc.sync.dma_start(qt0[:, 0:512], qt_d[0, :, 0:512])
        nc.sync.dma_start(mask_sb[:, 0:1, :], mk_d[:, 0:1, :])
        nc.sync.dma_start(qt0[:, 512:CHUNK], qt_d[0, :, 512:CHUNK])
        nc.sync.dma_start(kt0[:, P:], kt_d[0, :, P:])
        nc.sync.dma_start(qt0[:, CHUNK:], qt_d[0, :, CHUNK:])
        nc.sync.dma_start(vp0[:], vp_d[0])
        head_tiles = {0: (qt0, kt0, vp0)}
        for g in range(1, NKT):
            c0 = g * P
            nc.sync.dma_start(mask_sb[:, g:g + 1, c0:], mk_d[:, g:g + 1, c0:])

        for h in range(HPC):
            qt, kt, vp = head_tiles.pop(h, None) or load_head(h)
            outp = o_psum.tile([D + 1, S], f32, tag="outp")
            pending_pv = []

            def emit_evac(b):
                # bank b of outp ([65, 512] f32) is fully accumulated ->
                # convert to fp16 in SBUF and ship; host divides num/den.
                s0, s1 = 512 * b, 512 * (b + 1)
                osb = o_pool.tile([D + 1, 512], fp16, tag="osb")
                if b % 2 == 0:
                    # Copy lives in the same ACT table as Exp: no table reload
                    nc.scalar.activation(osb[:], outp[:, s0:s1], Copy)
                else:
                    nc.vector.tensor_copy(osb[:], outp[:, s0:s1])
                nc.sync.dma_start(out_d[h, :, s0:s1], osb[:])

            def emit_pv(j, c, e, p):
                for b in range(c // 512, (e + 511) // 512):
                    g0, g1 = max(c, 512 * b), min(e, 512 * (b + 1))
                    nc.tensor.matmul(outp[:, g0:g1], lhsT=vp[:, j, :],
                                     rhs=p[:, g0 - c:g1 - c],
                                     start=(j == 0),
                                     stop=(j == min(4 * b + 3, NKT - 1)))
                # bank b=(j-3)//4 is fully accumulated once k-tile j=4b+3's
                # last chunk (e == S) has been emitted
                if e == S and j % 4 == 3:
                    emit_evac((j - 3) // 4)

            def chunks(j):
                out, c = [], j * P
                while c < S:
                    e = min(S, (c // CHUNK + 1) * CHUNK)
                    out.append((c, e))
                    c = e
                return out

            t_idx = 0

            def emit_softmax(j, c, e, stt):
                nonlocal t_idx
                w = e - c
                p = p_pool.tile([P, CHUNK], fp16, tag="p")
                if SCHRAUD and t_idx % SCHRAUD == SCHRAUD // 2:
                    # full-DVE path: exp2 bit trick + fused mask multiply
                    isch = s_pool.tile([P, CHUNK], i16, tag="isch")
                    nc.vector.tensor_scalar(isch[:, :w], stt[:, :w],
                                            C_SCH, B_SCH, Mult, Add)
                    nc.vector.tensor_tensor(p[:, :w],
                                            isch[:, :w].bitcast(fp16),
                                            mask_sb[:, j, c:c + w], Mult)
                else:
                    nc.scalar.activation(p[:, :w], stt[:, :w], Exp, scale=0.125)
                    nc.vector.tensor_mul(p[:, :w], p[:, :w],
                                         mask_sb[:, j, c:c + w])
                t_idx += 1
                pending_pv.append((j, c, e, p))
                # drain PVs two at a time: each QK<->PV switch costs ~140ns
                # of PE row-config drain, so batching same-kind matmuls
                # halves the number of switches
                if len(pending_pv) >= PV_DELAY + 2:
                    emit_pv(*pending_pv.pop(0))
                    emit_pv(*pending_pv.pop(0))

            for j in range(NKT):
                # prefetch the next head's inputs mid-head so their DMAs
                # finish well before the head boundary (the tail of this
                # head still has ~8 k-tiles of PE work to hide them behind)
                if j == 8 and h + 1 < HPC:
                    head_tiles[h + 1] = load_head(h + 1)
                lhs = kt[:, j * P:(j + 1) * P]
                for c, e in chunks(j):
                    stt = st_psum.tile([P, CHUNK], f32, tag="st")
                    for lo in range(0, e - c, 512):
                        wl = min(512, e - c - lo)
                        nc.tensor.matmul(stt[:, lo:lo + wl], lhsT=lhs,
                                         rhs=qt[:, c + lo:c + lo + wl],
                                         start=True, stop=True)
                    emit_softmax(j, c, e, stt)

            while pending_pv:
                emit_pv(*pending_pv.pop(0))

    nc.compile()
    return nc


def prep_inputs(query, key, value, attn_mask):
    """Host-side layout prep (transposes/retiling/casts only) -> 8 in_maps."""
    query = np.asarray(query, dtype=np.float32)
    key = np.asarray(key, dtype=np.float32)
    value = np.asarray(value, dtype=np.float32)
    attn_mask = np.asarray(attn_mask).astype(bool)

    qT = query.transpose(0, 1, 3, 2).astype(np.float16)
    kT = key.transpose(0, 1, 3, 2).astype(np.float16)
    # zero-pad d-rows 64..127 (keeps the QK matmul in the PE's 128-row config)
    zpad = np.zeros((B, H, P - D, S), np.float16)
    qT = np.ascontiguousarray(np.concatenate([qT, zpad], axis=2))
    kT = np.ascontiguousarray(np.concatenate([kT, zpad], axis=2))

    vp = np.concatenate(
        [value, np.ones((B, H, S, 1), np.float32)], axis=3).astype(np.float16)
    # [B, H, S, 65] -> [B, H, 128, NKT, 65] (partition-contiguous tiles)
    vp = np.ascontiguousarray(
        vp.reshape(B, H, NKT, P, D + 1).transpose(0, 1, 3, 2, 4))

    tril = np.tril(np.ones((S, S), dtype=bool))
    in_maps = []
    for b in range(B):
        m = (attn_mask[b] & tril)          # [q, k]
        mT = m.T.astype(np.float16)        # [k, q]
        maskt = np.ascontiguousarray(
            mT.reshape(NKT, P, S).transpose(1, 0, 2))  # [128, NKT, S]
        for cl in range(NCORES // B):
            h0 = cl * HPC
            in_maps.append({
                "qt": np.ascontiguousarray(qT[b, h0:h0 + HPC]),
                "kt": np.ascontiguousarray(kT[b, h0:h0 + HPC]),
                "vp": np.ascontiguousarray(vp[b, h0:h0 + HPC]),
                "maskt": maskt,
            })
    return in_maps


def run(query, key, value, attn_mask, trace=False, trace_cores=None):
    from concourse import bass_utils

    if "nc" not in _cache:
        _cache["nc"] = build_nc()
    nc = _cache["nc"]

    in_maps = prep_inputs(query, key, value, attn_mask)
    res = bass_utils.run_bass_kernel_spmd(
        nc, in_maps, core_ids=list(range(NCORES)),
        trace=trace, trace_cores=trace_cores)

    out = np.empty((B, H, S, D), np.float32)
    for c in range(NCORES):
        b = c // (NCORES // B)
        h0 = (c % (NCORES // B)) * HPC
        outt = res.results[c]["outt"].astype(np.float32)   # [HPC, 65, S]
        num = outt[:, 0:D, :]                              # [HPC, 64, S]
        den = outt[:, D:D + 1, :]                          # [HPC, 1, S]
        out[b, h0:h0 + HPC] = (num / den).transpose(0, 2, 1)
    return out, res


def kernel(query, key, value, attn_mask):
    out, _ = run(query, key, value, attn_mask)
    return out
